# revision 16
# baseline (speedup 1.0000x reference)
"""Trainium2 Bass kernel for nn_DisentangledSelfAttention.

Sharding: batch (B=8) across the 8 NeuronCores, weights replicated.
Per core (one batch item, L=1024, E=1024, A=512, H=8, HD=64):

  xT = x.T (PE transpose)
  q0T/k0T/v0T = W_{Q,K,V}.T @ xT                 [E, L]   (lhsT = W natural)
  qT/kT = relu(Wq_w @ q0T + b)                   [A, L]   (lhsT = Wq_w.T via PE tp)
  v     = relu(v0.T_chunks @ Wv_w.T + b_row)     [L, A]   (natural layout)
  uT    = softmax_l(Wu_w @ k0T + bu)             [H, L]

  Group reshape (torch .view semantics): for group g (8 per batch item),
  pseudo-seq s = r*8 + c maps to (l = 128g + r, a = 64c + d).  We use a
  permuted order s' = c*16 + r (within each 128 k-chunk) / q' = c*64 +
  r_local (within each 512 q-chunk), consistently on the k-side, q-side and
  v-side, so the attention math is invariant.  Group-layout tensors
  (qT_store/kT_store [64,8,1024], vg [128,65], ucol [128,1]) are assembled
  via strided DRAM round-trip DMAs (engines cannot cross partitions).

  Per group: center q/k over s (free-dim mean); ST = mu_k.T @ mu_q (PE,
  K=64); PT = exp(ST/8) (ACT, PSUM->SBUF); outT = [v|1].T @ PT accumulated
  over k-chunks (row 64 = softmax denominators); uwv = ucol.T @ v; rank-1
  update outT += uwv.T @ s_row folds the unary row-bias through the later
  1/s scaling; PE-transpose outT back to [q', d]; scale rows by 1/s; DMA to
  the output with the inverse index map.
"""

import os
import numpy as np

B, L, E, A, H, HD = 8, 1024, 1024, 512, 8, 64
G = 8          # groups per batch item
N_CORES = 8

F32R = os.environ.get("KERNEL_F32R", "0") == "1"


def _build_nc():
    from contextlib import ExitStack

    import concourse.bass as bass
    import concourse.tile as tile
    import concourse.mybir as mybir
    from concourse import bacc
    from concourse.masks import make_identity

    f32 = mybir.dt.float32
    X = mybir.AxisListType.X

    def mm(ap):
        return ap.bitcast(mybir.dt.float32r) if F32R else ap

    nc = bacc.Bacc("TRN2", target_bir_lowering=False, debug=False,
                   num_devices=N_CORES)

    x_d = nc.dram_tensor("x", [L, E], f32, kind="ExternalInput").ap()
    WQ_d = nc.dram_tensor("W_Q", [E, E], f32, kind="ExternalInput").ap()
    WK_d = nc.dram_tensor("W_K", [E, E], f32, kind="ExternalInput").ap()
    WV_d = nc.dram_tensor("W_V", [E, E], f32, kind="ExternalInput").ap()
    Wq_w_d = nc.dram_tensor("Wq_w", [A, E], f32, kind="ExternalInput").ap()
    Wk_w_d = nc.dram_tensor("Wk_w", [A, E], f32, kind="ExternalInput").ap()
    Wv_w_d = nc.dram_tensor("Wv_w", [A, E], f32, kind="ExternalInput").ap()
    Wq_b_d = nc.dram_tensor("Wq_b", [A], f32, kind="ExternalInput").ap()
    Wk_b_d = nc.dram_tensor("Wk_b", [A], f32, kind="ExternalInput").ap()
    Wv_b_d = nc.dram_tensor("Wv_b", [A], f32, kind="ExternalInput").ap()
    Wu_w_d = nc.dram_tensor("Wu_w", [H, E], f32, kind="ExternalInput").ap()
    Wu_b_d = nc.dram_tensor("Wu_b", [H], f32, kind="ExternalInput").ap()
    out_d = nc.dram_tensor("out", [L, A], f32, kind="ExternalOutput").ap()

    with tile.TileContext(nc) as tc, ExitStack() as ctx:
        persist = ctx.enter_context(tc.tile_pool(name="persist", bufs=1))
        dram = ctx.enter_context(tc.tile_pool(name="dram", bufs=1, space="DRAM"))

        id128 = persist.tile([128, 128], f32, tag="id128")
        make_identity(nc, id128)
        id64 = persist.tile([64, 64], f32, tag="id64")
        make_identity(nc, id64)
        id8 = persist.tile([8, 8], f32, tag="id8")
        make_identity(nc, id8)
        id1 = persist.tile([1, 1], f32, tag="id1")
        make_identity(nc, id1)
        ones_row = persist.tile([1, 128], f32, tag="ones_row")
        nc.vector.memset(ones_row, 1.0)

        bq = persist.tile([128, 4], f32, tag="bq")
        nc.sync.dma_start(bq, Wq_b_d.rearrange("(j p) -> p j", p=128))
        bk = persist.tile([128, 4], f32, tag="bk")
        nc.sync.dma_start(bk, Wk_b_d.rearrange("(j p) -> p j", p=128))
        bv_row = persist.tile([1, 512], f32, tag="bv_row")
        nc.sync.dma_start(bv_row, Wv_b_d.rearrange("(one a) -> one a", one=1))
        bu = persist.tile([8, 1], f32, tag="bu")
        nc.sync.dma_start(bu, Wu_b_d.rearrange("(p one) -> p one", one=1))

        qT_dram = dram.tile([A, L], f32, tag="qT_dram")
        kT_dram = dram.tile([A, L], f32, tag="kT_dram")
        v_dram = dram.tile([L, A], f32, tag="v_dram")
        u_dram = dram.tile([H, L], f32, tag="u_dram")

        # =================== PHASE A ===================
        with tc.tile_pool(name="xT", bufs=1) as xT_pool, \
             tc.tile_pool(name="a_sb", bufs=2) as a_sb, \
             tc.tile_pool(name="w_sb", bufs=1) as w_pool, \
             tc.tile_pool(name="wraw", bufs=1) as wraw_pool, \
             tc.tile_pool(name="stage", bufs=1) as stage_pool, \
             tc.tile_pool(name="p0T", bufs=1) as p0T_pool, \
             tc.tile_pool(name="wt_sb", bufs=1) as wt_pool, \
             tc.tile_pool(name="a_mm", bufs=3, space="PSUM") as a_mm, \
             tc.tile_pool(name="a_tp", bufs=3, space="PSUM") as a_tp:

            # ---- x -> xT ----
            xT_all = xT_pool.tile([128, 8, 1024], f32, tag="xT_all")
            for i in range(8):          # l chunk
                xs = a_sb.tile([128, 1024], f32, tag="x_stage")
                nc.sync.dma_start(xs, x_d[128 * i:128 * i + 128, :])
                for j in range(8):      # e chunk
                    pt = a_tp.tile([128, 128], f32, tag="tp")
                    nc.tensor.transpose(pt, xs[:, 128 * j:128 * j + 128], id128)
                    nc.vector.tensor_copy(
                        out=xT_all[:, j, 128 * i:128 * i + 128], in_=pt)

            def big_proj(W_d, p0T_all):
                # p0T = W.T @ xT   [f, l]
                wsb = w_pool.tile([128, 8, 1024], f32, tag="wsb")
                nc.sync.dma_start(wsb, W_d.rearrange("(ec p) f -> p ec f", p=128))
                for fc in range(8):
                    for lc in range(2):
                        ps = a_mm.tile([128, 512], f32, tag="mm")
                        for ec in range(8):
                            nc.tensor.matmul(
                                ps,
                                mm(wsb[:, ec, 128 * fc:128 * fc + 128]),
                                mm(xT_all[:, ec, 512 * lc:512 * lc + 512]),
                                start=(ec == 0), stop=(ec == 7))
                        nc.any.tensor_copy(
                            out=p0T_all[:, fc, 512 * lc:512 * lc + 512], in_=ps)

            def load_wT(Ww_d):
                # Ww [A, E] -> wT_all[f_in, fc, a] = Ww.T chunks
                wT_all = wt_pool.tile([128, 8, 512], f32, tag="wT_all")
                wraw = wraw_pool.tile([128, 4, 1024], f32, tag="wraw")
                nc.sync.dma_start(wraw, Ww_d.rearrange("(ac p) f -> p ac f", p=128))
                for ac in range(4):
                    for fc in range(8):
                        pt = a_tp.tile([128, 128], f32, tag="tp")
                        nc.tensor.transpose(
                            pt, wraw[:, ac, 128 * fc:128 * fc + 128], id128)
                        nc.vector.tensor_copy(
                            out=wT_all[:, fc, 128 * ac:128 * ac + 128], in_=pt)
                return wT_all

            def qk_chain(p0T_all, wT_all, bias_col, dst_dram):
                # relu(Ww @ p0T + b) [A, L] -> staging -> one DMA to dram
                # (single-writer DRAM keeps downstream reload waits small)
                st = stage_pool.tile([128, 4, 1024], f32, tag="qk_stage")
                for ac in range(4):
                    for lc in range(2):
                        ps = a_mm.tile([128, 512], f32, tag="mm")
                        for fc in range(8):
                            nc.tensor.matmul(
                                ps,
                                mm(wT_all[:, fc, 128 * ac:128 * ac + 128]),
                                mm(p0T_all[:, fc, 512 * lc:512 * lc + 512]),
                                start=(fc == 0), stop=(fc == 7))
                        nc.scalar.activation(
                            out=st[:, ac, 512 * lc:512 * lc + 512], in_=ps,
                            func=mybir.ActivationFunctionType.Relu,
                            bias=bias_col[:, ac:ac + 1], scale=1.0)
                nc.sync.dma_start(
                    dst_dram[:].rearrange("(ac p) l -> p ac l", p=128), st)

            # ---- Q chain ----
            q0T_all = p0T_pool.tile([128, 8, 1024], f32, tag="p0T")
            big_proj(WQ_d, q0T_all)
            qk_chain(q0T_all, load_wT(Wq_w_d), bq, qT_dram)

            # ---- K chain ----
            k0T_all = p0T_pool.tile([128, 8, 1024], f32, tag="p0T")
            big_proj(WK_d, k0T_all)
            qk_chain(k0T_all, load_wT(Wk_w_d), bk, kT_dram)

            # ---- unary from k0T ----
            wu_sb = a_sb.tile([8, 1024], f32, tag="wu_sb")
            nc.sync.dma_start(wu_sb, Wu_w_d)
            wuT_all = wt_pool.tile([128, 8, 8], f32, tag="wuT_all")
            for fc in range(8):
                pt = a_tp.tile([128, 8], f32, tag="tp")
                nc.tensor.transpose(pt, wu_sb[:, 128 * fc:128 * fc + 128], id8)
                nc.vector.tensor_copy(out=wuT_all[:, fc, :], in_=pt)
            Ue = a_sb.tile([8, 1024], f32, tag="Ue")
            usum = a_sb.tile([8, 2], f32, tag="usum")
            for lc in range(2):
                psu = a_mm.tile([8, 512], f32, tag="mm")
                for fc in range(8):
                    nc.tensor.matmul(
                        psu, mm(wuT_all[:, fc, :]),
                        mm(k0T_all[:, fc, 512 * lc:512 * lc + 512]),
                        start=(fc == 0), stop=(fc == 7))
                nc.scalar.activation(
                    out=Ue[:, 512 * lc:512 * lc + 512], in_=psu,
                    func=mybir.ActivationFunctionType.Exp,
                    bias=bu, scale=1.0,
                    accum_out=usum[:, lc:lc + 1])
            ur = a_sb.tile([8, 1], f32, tag="ur")
            nc.vector.tensor_add(ur, usum[:, 0:1], usum[:, 1:2])
            nc.vector.reciprocal(out=ur, in_=ur)
            nc.vector.tensor_scalar_mul(Ue, Ue, ur)
            nc.sync.dma_start(u_dram, Ue)

            # ---- V chain (natural layout) ----
            v0T_all = p0T_pool.tile([128, 8, 1024], f32, tag="p0T")
            big_proj(WV_d, v0T_all)
            wvT = load_wT(Wv_w_d)
            v_all = stage_pool.tile([128, 8, 512], f32, tag="v_stage")
            for lt in range(8):
                ps = a_mm.tile([128, 512], f32, tag="mm")
                for fc in range(8):
                    nc.tensor.matmul(
                        ps, mm(v0T_all[:, fc, 128 * lt:128 * lt + 128]),
                        mm(wvT[:, fc, :]),
                        start=(fc == 0), stop=False)
                nc.tensor.matmul(ps, mm(ones_row), mm(bv_row),
                                 start=False, stop=True)
                nc.vector.tensor_scalar_max(v_all[:, lt, :], ps, 0.0)
            nc.sync.dma_start(
                v_dram[:].rearrange("(lt p) a -> p lt a", p=128), v_all)

        # =================== PHASE B ===================
        tc.strict_bb_all_engine_barrier()
        with tc.tile_pool(name="gstore", bufs=1) as gstore, \
             tc.tile_pool(name="pt_sb", bufs=24) as pt_pool, \
             tc.tile_pool(name="b_sb", bufs=4) as b_sb, \
             tc.tile_pool(name="b_small", bufs=4) as b_small, \
             tc.tile_pool(name="b_pair", bufs=2, space="PSUM") as b_pair, \
             tc.tile_pool(name="b_outT", bufs=2, space="PSUM") as b_outT, \
             tc.tile_pool(name="b_uwv", bufs=1, space="PSUM") as b_uwv, \
             tc.tile_pool(name="b_rc", bufs=1, space="PSUM") as b_rc, \
             tc.tile_pool(name="b_tp", bufs=2, space="PSUM") as b_tp:

            # group-ready layouts. k-chunk t = head-column c==t (s'' = r within
            # chunk); q-chunk h = c in [4h, 4h+4), q' = (c-4h)*128 + r.  Each
            # slice below has a single writer DMA (wait-count limits).
            qT_store = gstore.tile([64, G, 2, 512], f32, tag="qT_store")
            kT_store = gstore.tile([64, G, 8, 128], f32, tag="kT_store")
            vg_store = gstore.tile([128, G, 8, 65], f32, tag="vg_store")
            ucol_store = gstore.tile([128, G, 8], f32, tag="ucol_store")

            nc.vector.memset(vg_store[:, :, :, 64:65], 1.0)
            v_scr = v_dram[:].rearrange("(g r) (t d) -> t r g d",
                                        g=G, r=128, t=8, d=64)
            for t in range(8):
                nc.sync.dma_start(vg_store[:, :, t, 0:64], v_scr[t])
            u_scr = u_dram[:].rearrange("t (g r) -> t r g", g=G, r=128)
            for t in range(8):
                nc.sync.dma_start(ucol_store[:, :, t], u_scr[t])
            nc.sync.dma_start(
                qT_store.rearrange("d g h (cl r) -> d g h cl r", cl=4),
                qT_dram[:].rearrange("(h cl d) (g r) -> d g h cl r",
                                     h=2, cl=4, d=64, g=G, r=128))
            nc.sync.dma_start(
                kT_store.rearrange("d g t r -> d g t r"),
                kT_dram[:].rearrange("(t d) (g r) -> d g t r",
                                     t=8, d=64, g=G, r=128))

            inv_s = 1.0 / 1024.0
            for g in range(G):
                qg = qT_store[:, g].rearrange("d h q -> d (h q)")   # [64, 1024]
                kg = kT_store[:, g].rearrange("d t s -> d (t s)")
                for t_ap in (qg, kg):
                    mean = b_small.tile([64, 1], f32, tag="mean")
                    nc.vector.reduce_sum(mean, t_ap, axis=X)
                    nc.vector.tensor_scalar_mul(mean, mean, inv_s)
                    nc.vector.tensor_scalar_sub(t_ap, t_ap, mean)

                # uwv row [1, 64]
                ps_uwv = b_uwv.tile([1, 64], f32, tag="uwv")
                for t in range(8):
                    nc.tensor.matmul(
                        ps_uwv,
                        mm(ucol_store[:, g, t:t + 1]),
                        mm(vg_store[:, g, t, 0:64]),
                        start=(t == 0), stop=(t == 7))
                uwv_row = b_small.tile([1, 65], f32, tag="uwv_row")
                nc.vector.memset(uwv_row[:, 64:65], 0.0)
                nc.vector.tensor_copy(out=uwv_row[:, 0:64], in_=ps_uwv)

                for h in range(2):
                    rhs_q = qT_store[:, g, h]                  # [64, 512]
                    ps_outT = b_outT.tile([65, 512], f32, tag="outT")
                    for t in range(8):
                        lhs_k = kT_store[:, g, t]              # [64, 128]
                        ps_S = b_pair.tile([128, 512], f32, tag="pair")
                        nc.tensor.matmul(ps_S, mm(lhs_k), mm(rhs_q),
                                         start=True, stop=True)
                        pt_t = pt_pool.tile([128, 512], f32, tag="pt")
                        nc.scalar.activation(
                            out=pt_t, in_=ps_S,
                            func=mybir.ActivationFunctionType.Exp,
                            scale=0.125)
                        nc.tensor.matmul(ps_outT,
                                         mm(vg_store[:, g, t, :]),
                                         mm(pt_t),
                                         start=(t == 0), stop=(t == 7))
                    s_row = b_small.tile([1, 512], f32, tag="s_row")
                    nc.vector.tensor_copy(out=s_row, in_=ps_outT[64:65, :])
                    r_row = b_small.tile([1, 512], f32, tag="r_row")
                    nc.vector.reciprocal(out=r_row, in_=s_row)
                    nc.tensor.matmul(ps_outT, mm(uwv_row), mm(s_row),
                                     start=False, stop=True,
                                     skip_group_check=True)
                    sb_outT = b_sb.tile([64, 512], f32, tag="sb_outT")
                    nc.vector.tensor_copy(out=sb_outT, in_=ps_outT[0:64, :])
                    ps_rc = b_rc.tile([128, 4], f32, tag="rc")
                    for u in range(4):
                        nc.tensor.transpose(
                            ps_rc[:, u:u + 1],
                            r_row[:, 128 * u:128 * u + 128], id1)
                    rc_sb = b_small.tile([128, 4], f32, tag="rc_sb")
                    nc.vector.tensor_copy(out=rc_sb, in_=ps_rc)
                    for u in range(4):
                        ps_T = b_tp.tile([128, 64], f32, tag="fin_tp")
                        nc.tensor.transpose(
                            ps_T, sb_outT[:, 128 * u:128 * u + 128], id64)
                        ob = b_sb.tile([128, 64], f32, tag="ob")
                        nc.vector.tensor_scalar_mul(ob, ps_T, rc_sb[:, u:u + 1])
                        cc = 4 * h + u
                        nc.sync.dma_start(
                            out_d[128 * g:128 * g + 128,
                                  64 * cc:64 * cc + 64], ob)
    nc.compile()
    return nc


_NC_CACHE = {}


def kernel(**inputs):
    from concourse.bass_utils import run_bass_kernel_spmd

    if "nc" not in _NC_CACHE:
        _NC_CACHE["nc"] = _build_nc()
    nc = _NC_CACHE["nc"]

    x = np.ascontiguousarray(np.asarray(inputs["x"], dtype=np.float32))
    weights = {k: np.ascontiguousarray(np.asarray(v, dtype=np.float32))
               for k, v in inputs.items() if k != "x"}
    in_maps = [dict(weights, x=x[b]) for b in range(N_CORES)]

    trace = os.environ.get("KERNEL_TRACE", "0") == "1"
    res = run_bass_kernel_spmd(nc, in_maps, core_ids=list(range(N_CORES)),
                               trace=trace)
    if trace and res.exec_time_ns is not None:
        print(f"HW exec time: {res.exec_time_ns} ns")
        kernel.last_exec_time_ns = res.exec_time_ns
    out = np.stack([r["out"] for r in res.results], axis=0)
    return out
